# revision 7
# baseline (speedup 1.0000x reference)
"""GQA attention with LoRA-Q, tensor-parallel over 8 TRN2 cores — v5.

v11: PV accumulation restructured — hardware supports only ONE open matmul
accumulation group per PSUM bank at a time (interleaved groups corrupt), so
each query-subtile's PV accumulation now runs contiguously after the head's
exps, with the head's pt tiles buffered; the PV phase of head h interleaves
into the ST phase of head h+1.

Other refinements (from v5):
  - chunk c's O-assembly/out-projection ("tail") is split in two halves and
    interleaved into chunk c+1's first two attention-head slots, so tail PE
    work fills the Act-bound exp stretches;
  - diagonal exp pairs merged into single activations (the extra exp'd
    region is never read by PV);
  - triangular masks on the Pool engine (otherwise idle);
  - split first weight load + reordered prologue DMAs (PE starts ~4us);
  - kT2-dup DMAs placed so the SP queue never head-of-line blocks.
"""

import numpy as np
import ml_dtypes

import concourse.bass as bass
import concourse.mybir as mybir
import concourse.tile as tile
from concourse import bacc
from concourse.bass_utils import run_bass_kernel_spmd
from concourse.masks import make_identity

BF16 = mybir.dt.bfloat16
F32 = mybir.dt.float32

N_CORES = 8
T = 2048
D = 2048
HD = 64
NH = 32
NKV = 8
NH_LOC = NH // N_CORES
QW = NH_LOC * HD
P = 128
KT = D // P
CH = 512
NCH = T // CH
NJ = T // P
SCALE = 1.0 / 8.0
TPC = CH // P
OC = D // CH

TRI_ENGINE = "vector"   # "pool" | "vector"


def build_bass():
    nc = bacc.Bacc(None, num_devices=N_CORES)

    xT_d = nc.dram_tensor("xT", [D, T], BF16, kind="ExternalInput")
    w_d = nc.dram_tensor("w_all", [D, QW + 2 * HD], BF16, kind="ExternalInput")
    woT2_d = nc.dram_tensor("woT2", [P, 2, D], BF16, kind="ExternalInput")
    cos2_d = nc.dram_tensor("cos2", [P, T], BF16, kind="ExternalInput")
    sin2_d = nc.dram_tensor("sin2", [P, T], BF16, kind="ExternalInput")
    perm_d = nc.dram_tensor("perm", [P, P], BF16, kind="ExternalInput")
    tri_d = nc.dram_tensor("tri", [P, P], BF16, kind="ExternalInput")
    y_d = nc.dram_tensor("y", [T // N_CORES, D], BF16, kind="ExternalOutput")

    with tile.TileContext(nc, num_cores=N_CORES) as tc:
        _body(nc, tc, xT_d, w_d, woT2_d, cos2_d, sin2_d, perm_d, tri_d, y_d)
    nc.compile()
    return nc


def _body(nc, tc, xT_d, w_d, woT2_d, cos2_d, sin2_d, perm_d, tri_d, y_d):
    import contextlib

    ctx = contextlib.ExitStack()
    with ctx:
        consts = ctx.enter_context(tc.tile_pool(name="consts", bufs=1))
        big = ctx.enter_context(tc.tile_pool(name="big", bufs=1))
        work = ctx.enter_context(tc.tile_pool(name="work", bufs=1))
        projp = ctx.enter_context(tc.tile_pool(name="projp", bufs=2))
        ropep = ctx.enter_context(tc.tile_pool(name="ropep", bufs=3))
        ptp = ctx.enter_context(tc.tile_pool(name="ptp", bufs=15))
        o2p = ctx.enter_context(tc.tile_pool(name="o2p", bufs=2))
        rcpp = ctx.enter_context(tc.tile_pool(name="rcpp", bufs=3))
        ysp = ctx.enter_context(tc.tile_pool(name="ysp", bufs=2))
        psum_st = ctx.enter_context(tc.tile_pool(name="psum_st", bufs=2, space="PSUM"))
        psum_pv = ctx.enter_context(tc.tile_pool(name="psum_pv", bufs=2, space="PSUM"))
        psum_mm = ctx.enter_context(tc.tile_pool(name="psum_mm", bufs=2, space="PSUM"))
        dram = ctx.enter_context(tc.tile_pool(name="dram", bufs=1, space="DRAM"))

        # ---- prologue DMAs, ordered so chunk-0 projection starts earliest
        w_sb = consts.tile([P, KT, QW + 2 * HD], BF16)
        w_r = w_d.rearrange("(kt p) m -> p kt m", p=P)
        xT_sb = big.tile([P, KT, T], BF16)
        xT_r = xT_d.rearrange("(kt p) t -> p kt t", p=P)
        perm_sb = consts.tile([P, P], BF16)
        tri_sb = consts.tile([P, P], BF16)
        cos2_sb = consts.tile([P, T], BF16)
        sin2_sb = consts.tile([P, T], BF16)
        woT2_sb = consts.tile([P, 2, D], BF16)
        for kq in range(4):
            ks = slice(kq * KT // 4, (kq + 1) * KT // 4)
            nc.sync.dma_start(w_sb[:, ks, 0:P], w_r[:, ks, 0:P])
            nc.sync.dma_start(xT_sb[:, ks, 0:CH], xT_r[:, ks, 0:CH])
            if kq == 1:
                nc.sync.dma_start(perm_sb, perm_d[:])
                nc.sync.dma_start(cos2_sb[:, 0:CH], cos2_d[:, 0:CH])
                nc.sync.dma_start(sin2_sb[:, 0:CH], sin2_d[:, 0:CH])
                nc.sync.dma_start(tri_sb, tri_d[:])
        nc.sync.dma_start(w_sb[:, :, P:QW + 2 * HD], w_r[:, :, P:QW + 2 * HD])
        nc.sync.dma_start(xT_sb[:, 0:KT // 2, CH:2 * CH], xT_r[:, 0:KT // 2, CH:2 * CH])
        nc.sync.dma_start(xT_sb[:, KT // 2:KT, CH:2 * CH], xT_r[:, KT // 2:KT, CH:2 * CH])
        nc.sync.dma_start(cos2_sb[:, CH:T], cos2_d[:, CH:T])
        nc.sync.dma_start(sin2_sb[:, CH:T], sin2_d[:, CH:T])
        ident = consts.tile([P, P], BF16)
        make_identity(nc, ident)

        q2 = work.tile([P, 2, T], BF16)
        kT2 = work.tile([P, T], BF16)
        v_aug = work.tile([P, NJ, HD + 1], BF16)
        nc.vector.memset(v_aug[:, :, HD:HD + 1], 1.0)
        OT2 = work.tile([P, 2, T], BF16)

        y_dram = dram.tile([T, D], BF16)
        y_r = y_dram.rearrange("(ct p) d -> p ct d", p=P)
        y_rs = [dram.tile([HD, D], BF16, name=f"y_rs{c}") for c in range(NCH)]

        proj_tiles = {}
        o2_tiles = {}
        ysb_tiles = {}

        def proj_unit(c, m):
            csl = slice(c * CH, (c + 1) * CH)
            if m == 0:
                proj_tiles[c] = projp.tile([P, 3, CH], BF16, tag="projT",
                                           name=f"projT{c}")
            projT = proj_tiles[c]
            ps = psum_mm.tile([P, CH], F32, tag="mm")
            for kt in range(KT):
                nc.tensor.matmul(
                    ps,
                    lhsT=w_sb[:, kt, m * P:(m + 1) * P],
                    rhs=xT_sb[:, kt, csl],
                    start=(kt == 0),
                    stop=(kt == KT - 1),
                )
            if c < 2:
                nc.scalar.copy(projT[:, m, :], ps)   # Act idle in the lead
            else:
                nc.vector.tensor_copy(projT[:, m, :], ps)
            rot = psum_mm.tile([P, CH], F32, tag="mm")
            nc.tensor.matmul(rot, lhsT=perm_sb, rhs=projT[:, m, :],
                             start=True, stop=True)
            rows = slice(0, P if m < 2 else HD)
            t1 = ropep.tile([P, CH], BF16, tag="t1")
            nc.vector.tensor_mul(t1[rows, :], projT[rows, m, :], cos2_sb[rows, csl])
            t2 = ropep.tile([P, CH], BF16, tag="t2")
            nc.vector.tensor_mul(t2[rows, :], rot[rows, :], sin2_sb[rows, csl])
            if m < 2:
                nc.vector.tensor_add(q2[:, m, csl], t1, t2)
            else:
                nc.vector.tensor_add(kT2[0:HD, csl], t1[0:HD, :], t2[0:HD, :])
                for jj in range(TPC):
                    j = c * TPC + jj
                    vt = psum_mm.tile([P, HD], BF16, tag="mm")
                    nc.tensor.transpose(
                        vt, projT[HD:P, 2, jj * P:(jj + 1) * P],
                        ident[HD:P, HD:P],
                    )
                    nc.vector.tensor_copy(v_aug[:, j, 0:HD], vt)

        def kdup(c):
            csl = slice(c * CH, (c + 1) * CH)
            nc.sync.dma_start(kT2[HD:P, csl], kT2[0:HD, csl])

        pending2 = [None]

        def side_step():
            if pending2[0] is not None:
                try:
                    next(pending2[0])
                except StopIteration:
                    pending2[0] = None

        def drain_side():
            while pending2[0] is not None:
                side_step()

        def attn_phase2_gen(c, h, pts):
            """PV per query-subtile (contiguous accumulation groups — the hw
            allows only one open group per PSUM bank), then normalize."""
            o2 = o2_tiles[c]
            m = h // 2
            c4 = TPC * c
            pv = psum_pv.tile([P, TPC, HD + 1], F32, tag="pv")
            for s in range(TPC):
                for j in range(0, c4 + s + 1):
                    g, idx = divmod(j, 2)
                    nc.tensor.matmul(
                        pv[:, s, :],
                        lhsT=pts[g][:, idx, s * P:(s + 1) * P],
                        rhs=v_aug[:, j, :],
                        start=(j == 0),
                        stop=(j == c4 + s),
                        skip_group_check=True,
                    )
                yield
            rcp = rcpp.tile([P, TPC], F32, tag="rcp")
            nc.vector.reciprocal(rcp, pv[:, :, HD:HD + 1])
            for s in range(TPC):
                nc.vector.tensor_scalar_mul(
                    o2[:, s, m, (h % 2) * HD:(h % 2 + 1) * HD],
                    pv[:, s, 0:HD], rcp[:, s:s + 1])
            yield

        def attn_head(c, h):
            if h == 0:
                o2_tiles[c] = o2p.tile([P, TPC, 2, P], BF16, tag="o2",
                                       name=f"o2_{c}")
            m, b = h // 2, HD * (h % 2)
            c4 = TPC * c
            nj = c4 + TPC
            groups = [[g, g + 1] for g in range(0, nj, 2)]

            def do_st(js):
                st = psum_st.tile([P, 2, CH], F32, tag="st")
                for idx, j in enumerate(js):
                    lo = max(0, j - c4) * P
                    nc.tensor.matmul(
                        st[:, idx, lo:CH],
                        lhsT=kT2[b:b + HD, j * P:(j + 1) * P],
                        rhs=q2[b:b + HD, m, c * CH + lo:(c + 1) * CH],
                        start=True, stop=True,
                    )
                return st

            def do_exp(st, js):
                pt = ptp.tile([P, 2, CH], BF16, tag="pt")
                if js[0] >= c4:
                    lo = (js[0] - c4) * P
                    nc.scalar.activation(
                        pt[:, :, lo:CH], st[:, :, lo:CH],
                        mybir.ActivationFunctionType.Exp, scale=SCALE)
                    for idx, j in enumerate(js):
                        s0 = j - c4
                        teng = nc.gpsimd if TRI_ENGINE == "pool" else nc.vector
                        teng.tensor_mul(
                            pt[:, idx, s0 * P:(s0 + 1) * P],
                            pt[:, idx, s0 * P:(s0 + 1) * P], tri_sb)
                else:
                    nc.scalar.activation(
                        pt, st, mybir.ActivationFunctionType.Exp, scale=SCALE)
                return pt

            pts = []
            st_cur = do_st(groups[0])
            for g in range(len(groups)):
                st_next = do_st(groups[g + 1]) if g + 1 < len(groups) else None
                pts.append(do_exp(st_cur, groups[g]))
                side_step()
                side_step()
                st_cur = st_next
            drain_side()
            pending2[0] = attn_phase2_gen(c, h, pts)

        def tail_half(c, half):
            """O^T assembly + out-projection + y DMA for subtiles of `half`."""
            o2 = o2_tiles[c]
            if half == 0:
                ysb_tiles[c] = ysp.tile([P, TPC, D], BF16, tag="ysb",
                                        name=f"ysb{c}")
            y_sb = ysb_tiles[c]
            for s in (0, 1) if half == 0 else (2, 3):
                for g in range(2):
                    tp = psum_mm.tile([P, P], BF16, tag="mm")
                    nc.tensor.transpose(tp, o2[:, s, g, :], ident)
                    dst = OT2[:, g, c * CH + s * P:c * CH + (s + 1) * P]
                    if g == 0:
                        nc.vector.tensor_copy(dst, tp)
                    else:
                        nc.scalar.copy(dst, tp)
                for oc in range(OC):
                    ps = psum_mm.tile([P, CH], F32, tag="mm")
                    for g in range(2):
                        nc.tensor.matmul(
                            ps,
                            lhsT=OT2[:, g, c * CH + s * P:c * CH + (s + 1) * P],
                            rhs=woT2_sb[:, g, oc * CH:(oc + 1) * CH],
                            start=(g == 0), stop=(g == 1),
                        )
                    if c == NCH - 1 and oc % 2 == 1:
                        # terminal tail: Act is idle after the last exp
                        nc.scalar.copy(y_sb[:, s, oc * CH:(oc + 1) * CH], ps)
                    else:
                        nc.vector.tensor_copy(y_sb[:, s, oc * CH:(oc + 1) * CH], ps)
                    if c == NCH - 1:
                        nc.sync.dma_start(
                            y_r[:, TPC * c + s, oc * CH:(oc + 1) * CH],
                            y_sb[:, s, oc * CH:(oc + 1) * CH])
                if c != NCH - 1:
                    nc.sync.dma_start(y_r[:, TPC * c + s, :], y_sb[:, s, :])

        def rs(c):
            nc.gpsimd.collective_compute(
                "ReduceScatter",
                mybir.AluOpType.add,
                replica_groups=[list(range(N_CORES))],
                ins=[y_dram[c * CH:(c + 1) * CH, :]],
                outs=[y_rs[c].opt()],
            )
            nc.gpsimd.dma_start(y_d[c * HD:(c + 1) * HD, :], y_rs[c])

        # ---- interleaved emission
        # chunk 0: prologue proj, then heads with proj(1) fillers
        for m in range(3):
            proj_unit(0, m)
        kdup(0)
        nc.sync.dma_start(xT_sb[:, :, 2 * CH:3 * CH], xT_r[:, :, 2 * CH:3 * CH])
        proj_unit(1, 0)
        attn_head(0, 0)
        proj_unit(1, 1)
        attn_head(0, 2)
        proj_unit(1, 2)
        kdup(1)
        nc.sync.dma_start(woT2_sb, woT2_d[:])
        nc.sync.dma_start(xT_sb[:, :, 3 * CH:4 * CH], xT_r[:, :, 3 * CH:4 * CH])
        attn_head(0, 1)
        attn_head(0, 3)
        for c in range(1, NCH):
            if c > 1:
                kdup(c)
            attn_head(c, 0)
            tail_half(c - 1, 0)
            attn_head(c, 1)
            tail_half(c - 1, 1)
            rs(c - 1)
            attn_head(c, 2)
            if c + 1 < NCH:
                proj_unit(c + 1, 0)
            attn_head(c, 3)
            if c + 1 < NCH:
                proj_unit(c + 1, 1)
                proj_unit(c + 1, 2)
        drain_side()
        tail_half(NCH - 1, 0)
        tail_half(NCH - 1, 1)
        rs(NCH - 1)




def _prep_shards(x, Wq, lora_A, lora_B, Wk, Wv, Wo):
    bf16 = ml_dtypes.bfloat16
    xT = np.ascontiguousarray(x[0].T).astype(bf16)

    theta = 1.0 / (10000.0 ** (np.arange(0, HD, 2, dtype=np.float32) / HD))
    pos = np.arange(T, dtype=np.float32)
    ang = pos[:, None] * theta[None, :]
    ang = np.concatenate([ang, ang], axis=-1)
    cosT = np.cos(ang).T
    sinT = np.sin(ang).T
    cos2 = np.ascontiguousarray(np.concatenate([cosT, cosT], 0)).astype(bf16)
    sin2 = np.ascontiguousarray(np.concatenate([sinT, sinT], 0)).astype(bf16)

    M = np.zeros((HD, HD), np.float32)
    for i in range(HD // 2):
        M[i, i + HD // 2] = -1.0
        M[i + HD // 2, i] = 1.0
    perm = np.zeros((P, P), np.float32)
    perm[0:HD, 0:HD] = M.T
    perm[HD:P, HD:P] = M.T
    perm = perm.astype(bf16)

    tri = (np.arange(P)[:, None] <= np.arange(P)[None, :]).astype(bf16)

    Wq_eff = Wq + lora_B.astype(np.float64) @ lora_A.astype(np.float64)
    Wq_eff = Wq_eff.astype(np.float32)

    in_maps = []
    for i in range(N_CORES):
        wq_i = Wq_eff[QW * i:QW * (i + 1), :]
        wk_i = Wk[HD * i:HD * (i + 1), :]
        wv_i = Wv[HD * i:HD * (i + 1), :]
        w_all = np.ascontiguousarray(
            np.concatenate([wq_i, wk_i, wv_i], 0).T).astype(bf16)
        woT = Wo[:, QW * i:QW * (i + 1)].T
        woT2 = np.ascontiguousarray(
            woT.reshape(2, P, D).transpose(1, 0, 2)).astype(bf16)
        in_maps.append({
            "xT": xT,
            "w_all": w_all,
            "woT2": woT2,
            "cos2": cos2,
            "sin2": sin2,
            "perm": perm,
            "tri": tri,
        })
    return in_maps


def _unshard(results):
    y = np.zeros((T, D), np.float32)
    for i in range(N_CORES):
        yi = results[i]["y"].astype(np.float32)
        for c in range(NCH):
            y[c * CH + HD * i:c * CH + HD * (i + 1)] = yi[HD * c:HD * (c + 1)]
    return y[None]


def run(inputs, trace=False, **kw):
    nc = build_bass()
    in_maps = _prep_shards(**inputs)
    try:
        res = run_bass_kernel_spmd(
            nc, in_maps, core_ids=list(range(N_CORES)), trace=trace, **kw
        )
    except ModuleNotFoundError:
        # no NTFF profile hook in this environment -> run without trace
        res = run_bass_kernel_spmd(
            nc, in_maps, core_ids=list(range(N_CORES)), trace=False, **kw
        )
    return _unshard(res.results), res


def kernel(**inputs):
    y, _ = run(inputs)
    return y
